# revision 13
# baseline (speedup 1.0000x reference)
"""Multi-head attention (b=4, n=2048, dim=1024, heads=16, hd=64) on 8 TRN2
NeuronCores.

Sharding: core i = (batch b = i//2, head-half hh = i%2). Each core computes
Q/K/V projections for its 8 heads only (column-split QKV — no duplicated
K/V work), full 2048x2048 attention for those heads, and a row-split
out-projection partial; the host sums the two partials per batch and adds
the (bv-folded) output bias.

Device layouts (feature-major, partition dim first):
  xT   [128, 4 tc, 8 dc, 512]  x^T, d-chunked
  wq/wk [128, 4 fc, 8 dc, 128] fc-major so the critical fc0 slice is one DMA
  qT   [128, 4 fc, 2048 t]   Q^T local features (head pair p = chunk p)
  kT   [128, 4 fc, 2048 t]   K^T
  v    [128, 16 tt, 8 h, 64] V token-major per head
  S^T  psum [128 k, 2 h, 512 q] per k-tile: even head rows 0:64, odd 64:128
       of the PE array (tile_position row groups -> concurrent matmuls)
  P~   exp(S^T/8) bf16
  PV   po[128, 512] psum: even head d rows 0:64 (col groups 0-1), odd head
       rows 64:128 (col groups 2-3) -> the two M=64 matmuls run CONCURRENT
       in the PE's column tiles, halving PV cost vs the M=65 serial pair
  U    [128, 2, 512] bf16: running elementwise sum of P~ over k-tiles (DVE);
       softmax denominators = ones.T @ U via two tiny M=1 matmuls per unit
  attn [128, 4 fc, 2048 t]   normalized, head-concat feature-major
  outT [1024 e, 2048 t] bf16 partial (host sums core pairs in f32, adds bias)

Schedule: a stream of 16 (pair, q-chunk) units x 16 k-tile slots. Each slot
emits the two row-tiled score matmuls + exp + previous slot's PV, plus
"filler" projection matmuls popped from a deadline-ordered queue so the PE
never idles long enough for the HAM clock gate to re-throttle.

Normalization reshapes the 16 softmax-sum rows into [128, 8] (DRAM bounce)
before the reciprocal: DVE reciprocal is ~8 cyc/element *per partition lane*,
so a [2, 512] layout costs 4096 cycles while [128, 8] costs 64.
"""
import sys

sys.path.insert(0, "/opt/trn_rl_repo")

from collections import deque

import numpy as np
import ml_dtypes

import concourse.bass as bass
import concourse.tile as tile
from concourse import bacc, mybir
from concourse.bass_utils import run_bass_kernel_spmd

BF16 = mybir.dt.bfloat16
F32 = mybir.dt.float32
EXP = mybir.ActivationFunctionType.Exp
MULT = mybir.AluOpType.mult
ADD = mybir.AluOpType.add

D = 1024          # model dim
DC = 8            # d chunks of 128
NT = 2048         # tokens per core (q and k)
FL = 512          # local features (8 heads)
FC = 4            # local feature chunks of 128
NH = 8            # local heads
NP = 4            # local head pairs
HD = 64           # head dim
QC = 512          # q chunk (psum free)
NQC = 4           # q chunks
NKT = 16          # k tiles of 128
SB = 2            # heads per score psum tile (even/odd)
N_CORES = 8

_CACHE = {}


def _install_ntff_shim():
    """The agent image's ``antenv`` lacks ``axon_hooks``, so concourse's
    trace=True path can't find the NTFF profile hook even though
    ``libaxon_pjrt.so`` supports it. Recreate the glue (same contract as
    trn_boot's ``_ntff_profile_via_ctypes``)."""
    import types
    import ctypes
    import contextlib

    if "antenv.axon_hooks" in sys.modules:
        return
    so_path = "/opt/axon/libaxon_pjrt.so"
    try:
        lib = ctypes.CDLL(so_path)
        if not hasattr(lib, "axon_start_nrt_profile"):
            return
    except OSError:
        return
    lib.axon_start_nrt_profile.argtypes = [ctypes.POINTER(ctypes.c_int64),
                                           ctypes.c_size_t]
    lib.axon_start_nrt_profile.restype = ctypes.c_int64
    lib.axon_stop_nrt_profile.argtypes = [ctypes.c_char_p]
    lib.axon_stop_nrt_profile.restype = ctypes.c_int64

    @contextlib.contextmanager
    def _hook(output_dir, device_ids):
        import jax
        jax.devices()
        if device_ids:
            ids = (ctypes.c_int64 * len(device_ids))(*device_ids)
            rc = lib.axon_start_nrt_profile(ids, len(device_ids))
        else:
            rc = lib.axon_start_nrt_profile(None, 0)
        if rc != 0:
            raise RuntimeError(f"axon_start_nrt_profile rc={rc}")
        try:
            yield
        finally:
            n = lib.axon_stop_nrt_profile(str(output_dir).encode())
            print(f"ntff profile: {n} file(s) written to {output_dir}",
                  file=sys.stderr)

    mod = types.ModuleType("antenv.axon_hooks")
    _h = [_hook]
    mod.set_axon_ntff_profile_hook = lambda h: _h.__setitem__(0, h)
    mod.get_axon_ntff_profile_hook = lambda: _h[0]
    sys.modules["antenv.axon_hooks"] = mod
    import antenv
    antenv.axon_hooks = mod


def build():
    nc = bacc.Bacc("TRN2", target_bir_lowering=False, debug=False,
                   num_devices=N_CORES)

    xT_d = nc.dram_tensor("xT", [128, NQC, DC, QC], BF16,
                          kind="ExternalInput")
    wq_d = nc.dram_tensor("wqT", [128, FC, DC, 128], BF16,
                          kind="ExternalInput")
    wk_d = nc.dram_tensor("wkT", [128, FC, DC, 128], BF16,
                          kind="ExternalInput")
    wv_d = nc.dram_tensor("wvT", [128, DC, FL], BF16, kind="ExternalInput")
    ow_d = nc.dram_tensor("owT", [128, FC, D], BF16, kind="ExternalInput")
    bq_d = nc.dram_tensor("bq", [128, FC], F32, kind="ExternalInput")
    bk_d = nc.dram_tensor("bk", [128, FC], F32, kind="ExternalInput")
    out_d = nc.dram_tensor("outT", [D, NT], BF16, kind="ExternalOutput")

    with tile.TileContext(nc) as tc:
        with tc.tile_pool(name="persist", bufs=1) as persist:
            kT = persist.tile([128, FC, NT], BF16)
            qT = persist.tile([128, FC, NT], BF16)
            v = persist.tile([128, NKT, NH, HD], BF16)
            attn = persist.tile([128, FC, NT], BF16)
            bq_sb = persist.tile([128, FC], F32)
            bk_sb = persist.tile([128, FC], F32)
            one1 = persist.tile([128, 1], BF16)
            nc.vector.memset(one1, 1.0)
            warm = persist.tile([128, 1], F32)
            nc.vector.memset(warm, 0.0)

            # PSUM budget (8 banks): ps_acc 2x[128,512] proj/out/sums
            # accumulators, ps_s 2x[128,2,512] scores, ps_o 2x[128,512] PV.
            with tc.tile_pool(name="w1", bufs=1) as w1, \
                 tc.tile_pool(name="xpool", bufs=1) as xpool, \
                 tc.tile_pool(name="ppool", bufs=17) as ppool, \
                 tc.tile_pool(name="usum", bufs=2) as usum, \
                 tc.tile_pool(name="nrm", bufs=2) as nrm, \
                 tc.tile_pool(name="fout", bufs=3) as fout, \
                 tc.tile_pool(name="drpool", bufs=4, space="DRAM") as drpool, \
                 tc.tile_pool(name="ps_acc", bufs=2, space="PSUM") as ps_acc, \
                 tc.tile_pool(name="ps_s", bufs=2, space="PSUM") as ps_s, \
                 tc.tile_pool(name="ps_o", bufs=2, space="PSUM") as ps_o:
                xT = xpool.tile([128, NQC, DC, QC], BF16)
                wq = w1.tile([128, FC, DC, 128], BF16, tag="wq")
                wk = w1.tile([128, FC, DC, 128], BF16, tag="wk")
                wv = w1.tile([128, DC, FL], BF16, tag="wv")
                ow = w1.tile([128, FC, D], BF16, tag="ow")

                # Three queues in parallel, critical-first: the first K chain
                # needs only wk fc0 (one 256KB transfer) + xT tc0 (d-chunked,
                # progressive), then Q(0,0) needs wq fc0.
                # wk/wq fc0 lead the scalar queue (the biases' tiny 16B
                # partition lines are descriptor-churn; they are only needed
                # by the first chain's epilogue, several us later)
                nc.scalar.dma_start(out=wk[:, 0], in_=wk_d.ap()[:, 0])
                nc.scalar.dma_start(out=wq[:, 0], in_=wq_d.ap()[:, 0])
                nc.scalar.dma_start(out=bq_sb, in_=bq_d.ap())
                nc.scalar.dma_start(out=bk_sb, in_=bk_d.ap())
                # dummy exp pulls the ACT_TABLE_LOAD (~2.7us, blocks the
                # scalar sequencer) off the critical path — issued after the
                # critical weight DMAs so the table load overlaps transfers
                nc.scalar.activation(warm, warm, EXP)
                nc.sync.dma_start(out=xT[:, 0, 0:1, :],
                                  in_=xT_d.ap()[:, 0, 0:1, :])
                nc.sync.dma_start(out=xT[:, 0, 1:2, :],
                                  in_=xT_d.ap()[:, 0, 1:2, :])
                nc.sync.dma_start(out=xT[:, 0, 2:4, :],
                                  in_=xT_d.ap()[:, 0, 2:4, :])
                nc.sync.dma_start(out=xT[:, 0, 4:6, :],
                                  in_=xT_d.ap()[:, 0, 4:6, :])
                nc.sync.dma_start(out=xT[:, 0, 6:8, :],
                                  in_=xT_d.ap()[:, 0, 6:8, :])
                nc.scalar.dma_start(out=wk[:, 1], in_=wk_d.ap()[:, 1])
                nc.scalar.dma_start(out=wq[:, 1], in_=wq_d.ap()[:, 1])
                nc.gpsimd.dma_start(out=wv[:, 0:4, :], in_=wv_d.ap()[:, 0:4, :])
                nc.gpsimd.dma_start(out=wv[:, 4:8, :], in_=wv_d.ap()[:, 4:8, :])
                for h in (slice(0, 4), slice(4, 8)):
                    nc.sync.dma_start(out=xT[:, 1, h, :],
                                      in_=xT_d.ap()[:, 1, h, :])
                for fcw in (2, 3):
                    nc.gpsimd.dma_start(out=wk[:, fcw], in_=wk_d.ap()[:, fcw])
                    nc.gpsimd.dma_start(out=wq[:, fcw], in_=wq_d.ap()[:, fcw])
                nc.sync.dma_start(out=xT[:, 2, :, :], in_=xT_d.ap()[:, 2, :, :])
                nc.sync.dma_start(out=xT[:, 3, :, :], in_=xT_d.ap()[:, 3, :, :])
                nc.gpsimd.dma_start(out=ow, in_=ow_d.ap())

                # ---- projection chains (8 matmuls + epilogue each) ----
                def k_chain(fc, tc_i):
                    tsl = slice(tc_i * QC, (tc_i + 1) * QC)
                    ps = ps_acc.tile([128, QC], F32, tag="ps")
                    for dc in range(DC):
                        yield nc.tensor.matmul(
                            ps, lhsT=wk[:, fc, dc, :],
                            rhs=xT[:, tc_i, dc, :],
                            start=(dc == 0), stop=(dc == DC - 1))
                    yield nc.vector.tensor_scalar_add(
                        kT[:, fc, tsl], ps, bk_sb[:, fc:fc + 1])

                def q_chain(fc, tc_i):
                    tsl = slice(tc_i * QC, (tc_i + 1) * QC)
                    ps = ps_acc.tile([128, QC], F32, tag="ps")
                    for dc in range(DC):
                        yield nc.tensor.matmul(
                            ps, lhsT=wq[:, fc, dc, :],
                            rhs=xT[:, tc_i, dc, :],
                            start=(dc == 0), stop=(dc == DC - 1))
                    yield nc.vector.tensor_scalar_add(
                        qT[:, fc, tsl], ps, bq_sb[:, fc:fc + 1])

                def v_chain(tt):
                    ps = ps_acc.tile([128, QC], F32, tag="ps")
                    for dc in range(DC):
                        yield nc.tensor.matmul(
                            ps,
                            lhsT=xT[:, tt // 4, dc,
                                    (tt % 4) * 128:(tt % 4) * 128 + 128],
                            rhs=wv[:, dc, :],
                            start=(dc == 0), stop=(dc == DC - 1))
                    yield nc.vector.tensor_copy(
                        out=v[:, tt, :, :],
                        in_=ps.rearrange("p (h d) -> p h d", d=HD))

                def out_chain(ec, tc_i):
                    tsl = slice(tc_i * QC, (tc_i + 1) * QC)
                    ps = ps_acc.tile([128, QC], F32, tag="ps")
                    for fc in range(FC):
                        yield nc.tensor.matmul(
                            ps, lhsT=ow[:, fc, ec * 128:(ec + 1) * 128],
                            rhs=attn[:, fc, tsl],
                            start=(fc == 0), stop=(fc == FC - 1))
                    fo = fout.tile([128, QC], BF16, tag="fo")
                    # tc2/tc3 chains run in the tail where ACT is idle and
                    # DVE is busy with the final normalizations
                    if tc_i >= 2:
                        yield nc.scalar.activation(
                            fo, ps, mybir.ActivationFunctionType.Copy)
                    else:
                        yield nc.vector.tensor_copy(out=fo, in_=ps)
                    if tc_i == 3:
                        eng = (nc.gpsimd, nc.sync, nc.scalar)[ec % 3]
                    else:
                        eng = nc.gpsimd
                    yield eng.dma_start(
                        out=out_d.ap()[ec * 128:(ec + 1) * 128, tsl], in_=fo)

                # Deadline-ordered filler queue of (key, generator); attn
                # units pop a couple of steps per k-tile slot to keep the PE
                # dense while ACT owns the critical path.  Correctness rule:
                # everything a unit's own matmuls READ must be fully emitted
                # before the unit emits them — require() force-drains those.
                filler = deque()
                done_keys = set()

                def push(key, gen):
                    filler.append((key, gen))

                def drain(n):
                    for _ in range(n):
                        if not filler:
                            return
                        key, gen = filler[0]
                        try:
                            next(gen)
                        except StopIteration:
                            done_keys.add(key)
                            filler.popleft()

                def drain_all():
                    while filler:
                        drain(1)

                def require(*keys):
                    while any(k not in done_keys for k in keys):
                        assert filler, f"missing filler chains: {keys}"
                        drain(1)

                def attn_unit(p, qc, first=False, fill=2, extra=()):
                    # Cascaded schedule: EVERY unit defers its 16 PV matmul
                    # pairs + normalization into the NEXT unit's slots (the
                    # `extra` thunks, flushed three per slot so the previous
                    # unit's normalization fires ~10 slots before the unit
                    # boundary — out-proj chains gate on its attn write).
                    require(("k", p, 0), ("q", p, qc))
                    if not first:
                        require(*[("v", tt) for tt in range(NKT)])
                    he, ho = 2 * p, 2 * p + 1
                    qsl = slice(qc * QC, (qc + 1) * QC)
                    # one [128, 512] PSUM tile: even head d on partitions
                    # 0:64 (PE col groups 0-1), odd head on 64:128 (groups
                    # 2-3) -> the two PV matmuls execute concurrently
                    po = ps_o.tile([128, QC], F32, tag="po")
                    U = usum.tile([128, SB, QC], BF16, tag="U")

                    def pv(pt, kt):
                        nc.tensor.matmul(
                            po[0:HD, :], lhsT=v[:, kt, he, :], rhs=pt[:, 0, :],
                            start=(kt == 0), stop=(kt == NKT - 1))
                        nc.tensor.matmul(
                            po[HD:128, :], lhsT=v[:, kt, ho, :],
                            rhs=pt[:, 1, :],
                            start=(kt == 0), stop=(kt == NKT - 1))

                    extra = deque(extra)
                    backlog = []
                    for kt in range(NKT):
                        if kt % 4 == 0 and kt > 0:
                            require(("k", p, kt // 4))
                        ss = ps_s.tile([128, SB, QC], F32, tag="ss")
                        for j in range(SB):
                            hi = j * 64
                            nc.tensor.matmul(
                                ss[:, j, :],
                                lhsT=kT[hi:hi + HD, p,
                                        kt * 128:(kt + 1) * 128],
                                rhs=qT[hi:hi + HD, p, qsl],
                                start=True, stop=True)
                        pt = ppool.tile([128, SB, QC], BF16, tag="pt",
                                        bufs=19)
                        nc.scalar.activation(pt, ss, EXP, scale=0.125)
                        # running softmax-denominator accumulation (bf16,
                        # 2x DVE mode) — replaces the ones-column in V so
                        # the PV pair can column-tile at M=64
                        if kt == 0:
                            nc.vector.tensor_copy(out=U, in_=pt)
                        else:
                            nc.vector.tensor_tensor(out=U, in0=U, in1=pt,
                                                    op=ADD)
                        backlog.append((pt, kt))
                        for _ in range(3):
                            if extra:
                                extra.popleft()()
                        drain(fill)
                    while extra:
                        extra.popleft()()

                    cell = {}

                    def nsums():
                        cell["bc"] = _norm_sums(U)

                    def napply():
                        _norm_apply(p, qc, po, cell["bc"])

                    # norm_sums leads the backlog: the denominators depend
                    # only on U (complete at unit end), so its ~6us DMA
                    # chain (reshape -> reciprocal -> broadcast) runs under
                    # the PV flush instead of after it
                    return ([nsums] + [lambda a=a, b=b: pv(a, b)
                                       for a, b in backlog] + [napply])

                def _norm_sums(U):
                    # softmax denominators: ones.T @ U via two M=1 matmuls
                    # (psum rows 0 and 32 -> different PE column groups),
                    # then reshape to [128, 8] (DRAM bounce), reciprocal,
                    # DRAM-bounce the partition broadcast.
                    su = ps_acc.tile([128, QC], F32, tag="ps")
                    nc.tensor.matmul(su[0:1, :], lhsT=one1, rhs=U[:, 0, :],
                                     start=True, stop=True)
                    nc.tensor.matmul(su[32:33, :], lhsT=one1, rhs=U[:, 1, :],
                                     start=True, stop=True)
                    sums = nrm.tile([33, QC], BF16, tag="sums")
                    nc.vector.tensor_copy(out=sums, in_=su[0:33, :])
                    ds = drpool.tile([2, QC], BF16, tag="ds")
                    nc.sync.dma_start(out=ds[0:1, :], in_=sums[0:1, :])
                    nc.sync.dma_start(out=ds[1:2, :], in_=sums[32:33, :])
                    sr = nrm.tile([128, 8], BF16, tag="sr")
                    nc.sync.dma_start(
                        out=sr,
                        in_=bass.AP(tensor=ds.tensor, offset=ds.offset,
                                    ap=[[8, 128], [1, 8]]))
                    rc = nrm.tile([128, 8], BF16, tag="rc")
                    with nc.allow_low_precision(
                            reason="bf16 softmax denominators; rel-err "
                                   "budget 2e-2 >> bf16 rounding"):
                        nc.vector.reciprocal(rc, sr)
                    dr = drpool.tile([2, QC], BF16, tag="dr")
                    nc.sync.dma_start(
                        out=bass.AP(tensor=dr.tensor, offset=dr.offset,
                                    ap=[[8, 128], [1, 8]]),
                        in_=rc)
                    bc = nrm.tile([128, QC], BF16, tag="bc")
                    nc.sync.dma_start(
                        out=bc[0:HD, :],
                        in_=bass.AP(tensor=dr.tensor, offset=dr.offset,
                                    ap=[[0, HD], dr.ap[-1]]))
                    nc.sync.dma_start(
                        out=bc[HD:128, :],
                        in_=bass.AP(tensor=dr.tensor,
                                    offset=dr.offset + QC,
                                    ap=[[0, HD], dr.ap[-1]]))
                    return bc

                def _norm_apply(p, qc, po, bc):
                    # evacuate the PV accumulator and scale by the
                    # broadcast reciprocals: one full-width bf16 multiply
                    # writes both heads' attn directly
                    qsl = slice(qc * QC, (qc + 1) * QC)
                    pv_sb = nrm.tile([128, QC], BF16, tag="pv_sb", bufs=3)
                    nc.vector.tensor_copy(out=pv_sb, in_=po)
                    nc.vector.tensor_tensor(
                        out=attn[:, p, qsl], in0=pv_sb, in1=bc, op=MULT)

                # ---- emission ----
                # preamble: only K(0, tc0) + Q(0, qc0) gate the first scores
                push(("k", 0, 0), k_chain(0, 0))
                push(("q", 0, 0), q_chain(0, 0))
                require(("k", 0, 0), ("q", 0, 0))

                # unit 1's early fillers prefer tc0-data chains (xT tc1-3
                # are still in flight); V chains follow once wv lands
                push(("k", 1, 0), k_chain(1, 0))
                push(("q", 1, 0), q_chain(1, 0))
                for tc_i in range(1, NQC):
                    push(("k", 0, tc_i), k_chain(0, tc_i))
                for tt in range(NKT):
                    push(("v", tt), v_chain(tt))
                push(("q", 0, 1), q_chain(0, 1))
                for tc_i in range(1, NQC):
                    push(("k", 1, tc_i), k_chain(1, tc_i))
                bl = attn_unit(0, 0, first=True, fill=10)

                push(("q", 1, 1), q_chain(1, 1))
                push(("q", 0, 2), q_chain(0, 2))
                bl = attn_unit(0, 1, extra=bl, fill=3)
                push(("q", 0, 3), q_chain(0, 3))
                push(("q", 1, 2), q_chain(1, 2))
                bl = attn_unit(1, 0, extra=bl)
                for tc_i in range(NQC):
                    push(("k", 2, tc_i), k_chain(2, tc_i))
                bl = attn_unit(1, 1, extra=bl)
                push(("q", 1, 3), q_chain(1, 3))
                push(("q", 2, 0), q_chain(2, 0))
                bl = attn_unit(0, 2, extra=bl)
                push(("q", 2, 1), q_chain(2, 1))
                bl = attn_unit(0, 3, extra=bl)
                for tc_i in range(NQC):
                    push(("k", 3, tc_i), k_chain(3, tc_i))
                bl = attn_unit(1, 2, extra=bl)
                push(("q", 3, 0), q_chain(3, 0))
                push(("q", 3, 1), q_chain(3, 1))
                bl = attn_unit(1, 3, extra=bl)
                push(("q", 2, 2), q_chain(2, 2))
                push(("q", 2, 3), q_chain(2, 3))
                bl = attn_unit(2, 0, extra=bl)
                push(("q", 3, 2), q_chain(3, 2))
                push(("q", 3, 3), q_chain(3, 3))
                bl = attn_unit(2, 1, extra=bl)
                bl = attn_unit(3, 0, extra=bl)
                bl = attn_unit(3, 1, extra=bl)
                # qc0 attn for all pairs completes inside unit (3,1) (it
                # flushes (3,0)'s PV+norm) -> out-proj tc0 can follow
                for ec in range(DC):
                    push(("o", ec, 0), out_chain(ec, 0))
                bl = attn_unit(2, 2, extra=bl, fill=3)
                for ec in range(DC):
                    push(("o", ec, 1), out_chain(ec, 1))
                bl = attn_unit(2, 3, extra=bl, fill=3)
                bl = attn_unit(3, 2, extra=bl, fill=3)
                # fill=0 for the last unit: keep ALL remaining tc2 out-chain
                # steps to bridge the final normalization's latency window
                # (and the HAM clock gate) after the last exp
                bl = attn_unit(3, 3, extra=bl, fill=0)
                for ec in range(DC):
                    push(("o", ec, 2), out_chain(ec, 2))
                for t in bl:          # last unit's PV pairs back-to-back,
                    t()               # then its normalization immediately
                drain_all()           # tc2 chains bridge the norm latency
                for ec in range(DC):
                    push(("o", ec, 3), out_chain(ec, 3))
                drain_all()

    nc.compile()
    return nc


def _prep_in_maps(x, qkv_w, qkv_b, out_w, out_b):
    bf = ml_dtypes.bfloat16
    # xT: [1024 d, 2048 t] -> [128 p, 4 tc, 8 dc, 512] so each tc slice is
    # one contiguous-per-partition DMA
    xTs = []
    for b in range(4):
        xt = x[b].T.astype(bf)                       # [1024, 2048]
        xt = xt.reshape(DC, 128, NQC, QC).transpose(1, 2, 0, 3)
        xTs.append(np.ascontiguousarray(xt))
    wqT, wkT, wvT, owT, bq, bk = [], [], [], [], [], []
    for hh in range(2):
        fsl = slice(hh * FL, (hh + 1) * FL)
        # fc-major: [1024 in, 512 out] -> [128 p, 4 fc, 8 dc, 128 f]
        wqt = qkv_w[0:D][fsl].T.astype(bf)           # [1024 in, 512 out]
        wqT.append(np.ascontiguousarray(
            wqt.reshape(DC, 128, FC, 128).transpose(1, 2, 0, 3)))
        wkt = qkv_w[D:2 * D][fsl].T.astype(bf)
        wkT.append(np.ascontiguousarray(
            wkt.reshape(DC, 128, FC, 128).transpose(1, 2, 0, 3)))
        wvt = qkv_w[2 * D:3 * D][fsl].T.astype(bf)   # [1024 in, 512 out]
        wvT.append(np.ascontiguousarray(
            wvt.reshape(DC, 128, FL).transpose(1, 0, 2)))
        ow = out_w.T[fsl].astype(bf)                 # [512 f, 1024 e]
        owT.append(np.ascontiguousarray(
            ow.reshape(FC, 128, D).transpose(1, 0, 2)))
        bq.append(np.ascontiguousarray(
            qkv_b[0:D][fsl].reshape(FC, 128).T).astype(np.float32))
        bk.append(np.ascontiguousarray(
            qkv_b[D:2 * D][fsl].reshape(FC, 128).T).astype(np.float32))

    in_maps = []
    for i in range(N_CORES):
        b, hh = i // 2, i % 2
        in_maps.append(dict(xT=xTs[b], wqT=wqT[hh], wkT=wkT[hh],
                            wvT=wvT[hh], owT=owT[hh], bq=bq[hh], bk=bk[hh]))
    return in_maps


def run(x, qkv_w, qkv_b, out_w, out_b, trace=False):
    if trace:
        _install_ntff_shim()
    if "nc" not in _CACHE:
        _CACHE["nc"] = build()
    nc = _CACHE["nc"]
    x = np.asarray(x, np.float32)
    qkv_w = np.asarray(qkv_w, np.float32)
    qkv_b = np.asarray(qkv_b, np.float32)
    out_w = np.asarray(out_w, np.float32)
    out_b = np.asarray(out_b, np.float32)
    in_maps = _prep_in_maps(x, qkv_w, qkv_b, out_w, out_b)
    res = run_bass_kernel_spmd(nc, in_maps, core_ids=list(range(N_CORES)),
                               trace=trace)
    # host: sum the two head-half partials per batch, add bv-folded bias
    ob_eff = (out_b + out_w @ qkv_b[2 * D:3 * D]).astype(np.float32)
    out = np.empty((4, NT, D), np.float32)
    for b in range(4):
        acc = (res.results[2 * b]["outT"].astype(np.float32)
               + res.results[2 * b + 1]["outT"].astype(np.float32))
        out[b] = acc.T + ob_eff
    return out, res


def kernel(**inputs):
    out, _ = run(**inputs)
    return out


# revision 16
# speedup vs baseline: 1.0120x; 1.0120x over previous
"""Multi-head attention (b=4, n=2048, dim=1024, heads=16, hd=64) on 8 TRN2
NeuronCores.

Sharding: core i = (batch b = i//2, head-half hh = i%2). Each core computes
Q/K/V projections for its 8 heads only (column-split QKV — no duplicated
K/V work), full 2048x2048 attention for those heads, and a row-split
out-projection partial; the host sums the two partials per batch and adds
the (bv-folded) output bias.

Device layouts (feature-major, partition dim first):
  xT   [128, 4 tc, 8 dc, 512]  x^T, d-chunked
  wq/wk [128, 4 fc, 8 dc, 128] fc-major so the critical fc0 slice is one DMA
  qT   [128, 4 fc, 2048 t]   Q^T local features (head pair p = chunk p)
  kT   [128, 4 fc, 2048 t]   K^T
  v    [128, 16 tt, 8 h, 64] V token-major per head
  S^T  psum [128 k, 2 h, 512 q] per k-tile: even head rows 0:64, odd 64:128
       of the PE array (tile_position row groups -> concurrent matmuls)
  P~   exp(S^T/8) bf16
  PV   po[128, 512] psum: even head d rows 0:64 (col groups 0-1), odd head
       rows 64:128 (col groups 2-3) -> the two M=64 matmuls run CONCURRENT
       in the PE's column tiles, halving PV cost vs the M=65 serial pair
  U    [128, 2, 512] bf16: running elementwise sum of P~ over k-tiles (DVE);
       softmax denominators = ones.T @ U via two tiny M=1 matmuls per unit
  attn [128, 4 fc, 2048 t]   normalized, head-concat feature-major
  outT [1024 e, 2048 t] bf16 partial (host sums core pairs in f32, adds bias)

Schedule: a stream of 16 (pair, q-chunk) units x 16 k-tile slots. Each slot
emits the two row-tiled score matmuls + exp + previous slot's PV, plus
"filler" projection matmuls popped from a deadline-ordered queue so the PE
never idles long enough for the HAM clock gate to re-throttle.

Normalization reshapes the 16 softmax-sum rows into [128, 8] (DRAM bounce)
before the reciprocal: DVE reciprocal is ~8 cyc/element *per partition lane*,
so a [2, 512] layout costs 4096 cycles while [128, 8] costs 64.
"""
import sys

sys.path.insert(0, "/opt/trn_rl_repo")

from collections import deque

import numpy as np
import ml_dtypes

import concourse.bass as bass
import concourse.tile as tile
from concourse import bacc, mybir
from concourse.bass_utils import run_bass_kernel_spmd

BF16 = mybir.dt.bfloat16
F32 = mybir.dt.float32
EXP = mybir.ActivationFunctionType.Exp
MULT = mybir.AluOpType.mult
ADD = mybir.AluOpType.add

D = 1024          # model dim
DC = 8            # d chunks of 128
NT = 2048         # tokens per core (q and k)
FL = 512          # local features (8 heads)
FC = 4            # local feature chunks of 128
NH = 8            # local heads
NP = 4            # local head pairs
HD = 64           # head dim
QC = 512          # q chunk (psum free)
NQC = 4           # q chunks
NKT = 16          # k tiles of 128
SB = 2            # heads per score psum tile (even/odd)
N_CORES = 8

_CACHE = {}


def _install_ntff_shim():
    """The agent image's ``antenv`` lacks ``axon_hooks``, so concourse's
    trace=True path can't find the NTFF profile hook even though
    ``libaxon_pjrt.so`` supports it. Recreate the glue (same contract as
    trn_boot's ``_ntff_profile_via_ctypes``)."""
    import types
    import ctypes
    import contextlib

    if "antenv.axon_hooks" in sys.modules:
        return
    so_path = "/opt/axon/libaxon_pjrt.so"
    try:
        lib = ctypes.CDLL(so_path)
        if not hasattr(lib, "axon_start_nrt_profile"):
            return
    except OSError:
        return
    lib.axon_start_nrt_profile.argtypes = [ctypes.POINTER(ctypes.c_int64),
                                           ctypes.c_size_t]
    lib.axon_start_nrt_profile.restype = ctypes.c_int64
    lib.axon_stop_nrt_profile.argtypes = [ctypes.c_char_p]
    lib.axon_stop_nrt_profile.restype = ctypes.c_int64

    @contextlib.contextmanager
    def _hook(output_dir, device_ids):
        import jax
        jax.devices()
        if device_ids:
            ids = (ctypes.c_int64 * len(device_ids))(*device_ids)
            rc = lib.axon_start_nrt_profile(ids, len(device_ids))
        else:
            rc = lib.axon_start_nrt_profile(None, 0)
        if rc != 0:
            raise RuntimeError(f"axon_start_nrt_profile rc={rc}")
        try:
            yield
        finally:
            n = lib.axon_stop_nrt_profile(str(output_dir).encode())
            print(f"ntff profile: {n} file(s) written to {output_dir}",
                  file=sys.stderr)

    mod = types.ModuleType("antenv.axon_hooks")
    _h = [_hook]
    mod.set_axon_ntff_profile_hook = lambda h: _h.__setitem__(0, h)
    mod.get_axon_ntff_profile_hook = lambda: _h[0]
    sys.modules["antenv.axon_hooks"] = mod
    import antenv
    antenv.axon_hooks = mod


def build():
    nc = bacc.Bacc("TRN2", target_bir_lowering=False, debug=False,
                   num_devices=N_CORES)

    xT_d = nc.dram_tensor("xT", [128, NQC, DC, QC], BF16,
                          kind="ExternalInput")
    wq_d = nc.dram_tensor("wqT", [128, FC, DC, 128], BF16,
                          kind="ExternalInput")
    wk_d = nc.dram_tensor("wkT", [128, FC, DC, 128], BF16,
                          kind="ExternalInput")
    wv_d = nc.dram_tensor("wvT", [128, DC, FL], BF16, kind="ExternalInput")
    ow_d = nc.dram_tensor("owT", [128, FC, D], BF16, kind="ExternalInput")
    bq_d = nc.dram_tensor("bq", [128, FC], F32, kind="ExternalInput")
    bk_d = nc.dram_tensor("bk", [128, FC], F32, kind="ExternalInput")
    out_d = nc.dram_tensor("outT", [D, NT], BF16, kind="ExternalOutput")

    with tile.TileContext(nc) as tc:
        with tc.tile_pool(name="persist", bufs=1) as persist:
            kT = persist.tile([128, FC, NT], BF16)
            qT = persist.tile([128, FC, NT], BF16)
            v = persist.tile([128, NKT, NH, HD], BF16)
            attn = persist.tile([128, FC, NT], BF16)
            bq_sb = persist.tile([128, FC], F32)
            bk_sb = persist.tile([128, FC], F32)
            one1 = persist.tile([128, 1], BF16)
            nc.vector.memset(one1, 1.0)
            warm = persist.tile([128, 1], F32)
            nc.vector.memset(warm, 0.0)

            # PSUM budget (8 banks): ps_acc 2x[128,512] proj/out/sums
            # accumulators, ps_s 2x[128,2,512] scores, ps_o 2x[128,512] PV.
            with tc.tile_pool(name="w1", bufs=1) as w1, \
                 tc.tile_pool(name="xpool", bufs=1) as xpool, \
                 tc.tile_pool(name="ppool", bufs=17) as ppool, \
                 tc.tile_pool(name="usum", bufs=2) as usum, \
                 tc.tile_pool(name="nrm", bufs=2) as nrm, \
                 tc.tile_pool(name="fout", bufs=3) as fout, \
                 tc.tile_pool(name="drpool", bufs=4, space="DRAM") as drpool, \
                 tc.tile_pool(name="ps_acc", bufs=2, space="PSUM") as ps_acc, \
                 tc.tile_pool(name="ps_s", bufs=2, space="PSUM") as ps_s, \
                 tc.tile_pool(name="ps_o", bufs=2, space="PSUM") as ps_o:
                xT = xpool.tile([128, NQC, DC, QC], BF16)
                wq = w1.tile([128, FC, DC, 128], BF16, tag="wq")
                wk = w1.tile([128, FC, DC, 128], BF16, tag="wk")
                wv = w1.tile([128, DC, FL], BF16, tag="wv")
                ow = w1.tile([128, FC, D], BF16, tag="ow")

                # Three queues in parallel, critical-first: the first K chain
                # needs only wk fc0 (one 256KB transfer) + xT tc0 (d-chunked,
                # progressive), then Q(0,0) needs wq fc0.  The biases' tiny
                # 16B partition lines are pure descriptor churn and only
                # feed the first chain's epilogue — they queue behind.
                nc.scalar.dma_start(out=wk[:, 0], in_=wk_d.ap()[:, 0])
                nc.scalar.dma_start(out=wq[:, 0], in_=wq_d.ap()[:, 0])
                nc.scalar.dma_start(out=bq_sb, in_=bq_d.ap())
                nc.scalar.dma_start(out=bk_sb, in_=bk_d.ap())
                # dummy exp pulls the ACT_TABLE_LOAD (~2.7us, blocks the
                # scalar sequencer) off the critical path — issued after the
                # critical weight DMAs so the table load overlaps transfers
                nc.scalar.activation(warm, warm, EXP)
                nc.sync.dma_start(out=xT[:, 0, 0:1, :],
                                  in_=xT_d.ap()[:, 0, 0:1, :])
                nc.sync.dma_start(out=xT[:, 0, 1:2, :],
                                  in_=xT_d.ap()[:, 0, 1:2, :])
                nc.sync.dma_start(out=xT[:, 0, 2:4, :],
                                  in_=xT_d.ap()[:, 0, 2:4, :])
                nc.sync.dma_start(out=xT[:, 0, 4:6, :],
                                  in_=xT_d.ap()[:, 0, 4:6, :])
                nc.sync.dma_start(out=xT[:, 0, 6:8, :],
                                  in_=xT_d.ap()[:, 0, 6:8, :])
                nc.scalar.dma_start(out=wk[:, 1], in_=wk_d.ap()[:, 1])
                nc.scalar.dma_start(out=wq[:, 1], in_=wq_d.ap()[:, 1])
                nc.gpsimd.dma_start(out=wv[:, 0:4, :], in_=wv_d.ap()[:, 0:4, :])
                nc.gpsimd.dma_start(out=wv[:, 4:8, :], in_=wv_d.ap()[:, 4:8, :])
                for h in (slice(0, 4), slice(4, 8)):
                    nc.sync.dma_start(out=xT[:, 1, h, :],
                                      in_=xT_d.ap()[:, 1, h, :])
                for fcw in (2, 3):
                    nc.gpsimd.dma_start(out=wk[:, fcw], in_=wk_d.ap()[:, fcw])
                    nc.gpsimd.dma_start(out=wq[:, fcw], in_=wq_d.ap()[:, fcw])
                nc.sync.dma_start(out=xT[:, 2, :, :], in_=xT_d.ap()[:, 2, :, :])
                nc.sync.dma_start(out=xT[:, 3, :, :], in_=xT_d.ap()[:, 3, :, :])
                nc.gpsimd.dma_start(out=ow, in_=ow_d.ap())

                # PE warm-up: ~4us of dummy matmuls during the (DMA-bound)
                # input load releases the HAM clock gate (default 1.2 GHz,
                # 2.4 GHz after ~3.4us of sustained activity) before the
                # first projection chains run — they measured 427-609ns/MM
                # cold vs 216 warm, ~4-5us lost
                dummy = persist.tile([128, QC], BF16)
                nc.vector.memset(dummy, 0.0)
                wps = ps_acc.tile([128, QC], F32, tag="ps")
                for _ in range(9):
                    nc.tensor.matmul(wps[0:1, :], lhsT=one1, rhs=dummy,
                                     start=True, stop=True)

                # ---- projection chains (8 matmuls + epilogue each) ----
                def k_chain(fc, tc_i):
                    tsl = slice(tc_i * QC, (tc_i + 1) * QC)
                    ps = ps_acc.tile([128, QC], F32, tag="ps")
                    for dc in range(DC):
                        yield nc.tensor.matmul(
                            ps, lhsT=wk[:, fc, dc, :],
                            rhs=xT[:, tc_i, dc, :],
                            start=(dc == 0), stop=(dc == DC - 1))
                    yield nc.vector.tensor_scalar_add(
                        kT[:, fc, tsl], ps, bk_sb[:, fc:fc + 1])

                def q_chain(fc, tc_i):
                    tsl = slice(tc_i * QC, (tc_i + 1) * QC)
                    ps = ps_acc.tile([128, QC], F32, tag="ps")
                    for dc in range(DC):
                        yield nc.tensor.matmul(
                            ps, lhsT=wq[:, fc, dc, :],
                            rhs=xT[:, tc_i, dc, :],
                            start=(dc == 0), stop=(dc == DC - 1))
                    yield nc.vector.tensor_scalar_add(
                        qT[:, fc, tsl], ps, bq_sb[:, fc:fc + 1])

                def v_chain(tt):
                    ps = ps_acc.tile([128, QC], F32, tag="ps")
                    for dc in range(DC):
                        yield nc.tensor.matmul(
                            ps,
                            lhsT=xT[:, tt // 4, dc,
                                    (tt % 4) * 128:(tt % 4) * 128 + 128],
                            rhs=wv[:, dc, :],
                            start=(dc == 0), stop=(dc == DC - 1))
                    yield nc.vector.tensor_copy(
                        out=v[:, tt, :, :],
                        in_=ps.rearrange("p (h d) -> p h d", d=HD))

                def out_chain(ec, tc_i):
                    tsl = slice(tc_i * QC, (tc_i + 1) * QC)
                    ps = ps_acc.tile([128, QC], F32, tag="ps")
                    for fc in range(FC):
                        yield nc.tensor.matmul(
                            ps, lhsT=ow[:, fc, ec * 128:(ec + 1) * 128],
                            rhs=attn[:, fc, tsl],
                            start=(fc == 0), stop=(fc == FC - 1))
                    fo = fout.tile([128, QC], BF16, tag="fo")
                    # tc2/tc3 chains run in the tail where ACT is idle and
                    # DVE is busy with the final normalizations
                    if tc_i >= 2:
                        yield nc.scalar.activation(
                            fo, ps, mybir.ActivationFunctionType.Copy)
                    else:
                        yield nc.vector.tensor_copy(out=fo, in_=ps)
                    if tc_i == 3:
                        eng = (nc.gpsimd, nc.sync, nc.scalar)[ec % 3]
                    else:
                        eng = nc.gpsimd
                    yield eng.dma_start(
                        out=out_d.ap()[ec * 128:(ec + 1) * 128, tsl], in_=fo)

                # Deadline-ordered filler queue of (key, generator); attn
                # units pop a couple of steps per k-tile slot to keep the PE
                # dense while ACT owns the critical path.  Correctness rule:
                # everything a unit's own matmuls READ must be fully emitted
                # before the unit emits them — require() force-drains those.
                filler = deque()
                done_keys = set()

                def push(key, gen):
                    filler.append((key, gen))

                def drain(n):
                    for _ in range(n):
                        if not filler:
                            return
                        key, gen = filler[0]
                        try:
                            next(gen)
                        except StopIteration:
                            done_keys.add(key)
                            filler.popleft()

                def drain_all():
                    while filler:
                        drain(1)

                def require(*keys):
                    while any(k not in done_keys for k in keys):
                        assert filler, f"missing filler chains: {keys}"
                        drain(1)

                def attn_unit(p, qc, first=False, fill=2, extra=()):
                    # Cascaded schedule: EVERY unit defers its 16 PV matmul
                    # pairs + normalization into the NEXT unit's slots (the
                    # `extra` thunks, flushed three per slot so the previous
                    # unit's normalization fires ~10 slots before the unit
                    # boundary — out-proj chains gate on its attn write).
                    require(("k", p, 0), ("q", p, qc))
                    if not first:
                        require(*[("v", tt) for tt in range(NKT)])
                    he, ho = 2 * p, 2 * p + 1
                    qsl = slice(qc * QC, (qc + 1) * QC)
                    # one [128, 512] PSUM tile: even head d on partitions
                    # 0:64 (PE col groups 0-1), odd head on 64:128 (groups
                    # 2-3) -> the two PV matmuls execute concurrently
                    po = ps_o.tile([128, QC], F32, tag="po")
                    U = usum.tile([128, SB, QC], BF16, tag="U")

                    def pv(pt, kt):
                        nc.tensor.matmul(
                            po[0:HD, :], lhsT=v[:, kt, he, :], rhs=pt[:, 0, :],
                            start=(kt == 0), stop=(kt == NKT - 1))
                        nc.tensor.matmul(
                            po[HD:128, :], lhsT=v[:, kt, ho, :],
                            rhs=pt[:, 1, :],
                            start=(kt == 0), stop=(kt == NKT - 1))

                    extra = deque(extra)
                    backlog = []
                    for kt in range(NKT):
                        if kt % 4 == 0 and kt > 0:
                            require(("k", p, kt // 4))
                        ss = ps_s.tile([128, SB, QC], F32, tag="ss")
                        for j in range(SB):
                            hi = j * 64
                            nc.tensor.matmul(
                                ss[:, j, :],
                                lhsT=kT[hi:hi + HD, p,
                                        kt * 128:(kt + 1) * 128],
                                rhs=qT[hi:hi + HD, p, qsl],
                                start=True, stop=True)
                        pt = ppool.tile([128, SB, QC], BF16, tag="pt",
                                        bufs=19)
                        nc.scalar.activation(pt, ss, EXP, scale=0.125)
                        # running softmax-denominator accumulation (bf16,
                        # 2x DVE mode) — replaces the ones-column in V so
                        # the PV pair can column-tile at M=64
                        if kt == 0:
                            nc.vector.tensor_copy(out=U, in_=pt)
                        else:
                            nc.vector.tensor_tensor(out=U, in0=U, in1=pt,
                                                    op=ADD)
                        backlog.append((pt, kt))
                        for _ in range(3):
                            if extra:
                                extra.popleft()()
                        drain(fill)
                    while extra:
                        extra.popleft()()

                    def norm():
                        _norm(p, qc, po, U)

                    return ([lambda a=a, b=b: pv(a, b)
                             for a, b in backlog] + [norm])

                def _norm(p, qc, po, U):
                    # softmax denominators: ones.T @ U via two M=1 matmuls
                    # (psum rows 0 and 32 -> different PE column groups),
                    # then reshape to [128, 8] (DRAM bounce), reciprocal,
                    # DRAM-bounce the partition broadcast, one full-width
                    # bf16 multiply writes both heads' attn directly.
                    qsl = slice(qc * QC, (qc + 1) * QC)
                    su = ps_acc.tile([128, QC], F32, tag="ps")
                    nc.tensor.matmul(su[0:1, :], lhsT=one1, rhs=U[:, 0, :],
                                     start=True, stop=True)
                    nc.tensor.matmul(su[32:33, :], lhsT=one1, rhs=U[:, 1, :],
                                     start=True, stop=True)
                    sums = nrm.tile([33, QC], BF16, tag="sums")
                    nc.vector.tensor_copy(out=sums, in_=su[0:33, :])
                    pv_sb = nrm.tile([128, QC], BF16, tag="pv_sb", bufs=3)
                    nc.vector.tensor_copy(out=pv_sb, in_=po)
                    ds = drpool.tile([2, QC], BF16, tag="ds")
                    nc.sync.dma_start(out=ds[0:1, :], in_=sums[0:1, :])
                    nc.sync.dma_start(out=ds[1:2, :], in_=sums[32:33, :])
                    sr = nrm.tile([128, 8], BF16, tag="sr")
                    nc.sync.dma_start(
                        out=sr,
                        in_=bass.AP(tensor=ds.tensor, offset=ds.offset,
                                    ap=[[8, 128], [1, 8]]))
                    rc = nrm.tile([128, 8], BF16, tag="rc")
                    with nc.allow_low_precision(
                            reason="bf16 softmax denominators; rel-err "
                                   "budget 2e-2 >> bf16 rounding"):
                        nc.vector.reciprocal(rc, sr)
                    dr = drpool.tile([2, QC], BF16, tag="dr")
                    nc.sync.dma_start(
                        out=bass.AP(tensor=dr.tensor, offset=dr.offset,
                                    ap=[[8, 128], [1, 8]]),
                        in_=rc)
                    bc = nrm.tile([128, QC], BF16, tag="bc")
                    nc.sync.dma_start(
                        out=bc[0:HD, :],
                        in_=bass.AP(tensor=dr.tensor, offset=dr.offset,
                                    ap=[[0, HD], dr.ap[-1]]))
                    nc.sync.dma_start(
                        out=bc[HD:128, :],
                        in_=bass.AP(tensor=dr.tensor,
                                    offset=dr.offset + QC,
                                    ap=[[0, HD], dr.ap[-1]]))
                    nc.vector.tensor_tensor(
                        out=attn[:, p, qsl], in0=pv_sb, in1=bc, op=MULT)

                # ---- emission ----
                # preamble: only K(0, tc0) + Q(0, qc0) gate the first scores
                push(("k", 0, 0), k_chain(0, 0))
                push(("q", 0, 0), q_chain(0, 0))
                require(("k", 0, 0), ("q", 0, 0))

                # unit 1's early fillers prefer tc0-data chains (xT tc1-3
                # are still in flight); V chains follow once wv lands
                push(("k", 1, 0), k_chain(1, 0))
                push(("q", 1, 0), q_chain(1, 0))
                for tc_i in range(1, NQC):
                    push(("k", 0, tc_i), k_chain(0, tc_i))
                for tt in range(NKT):
                    push(("v", tt), v_chain(tt))
                push(("q", 0, 1), q_chain(0, 1))
                for tc_i in range(1, NQC):
                    push(("k", 1, tc_i), k_chain(1, tc_i))
                bl = attn_unit(0, 0, first=True, fill=10)

                push(("q", 1, 1), q_chain(1, 1))
                push(("q", 0, 2), q_chain(0, 2))
                bl = attn_unit(0, 1, extra=bl, fill=3)
                push(("q", 0, 3), q_chain(0, 3))
                push(("q", 1, 2), q_chain(1, 2))
                bl = attn_unit(1, 0, extra=bl)
                for tc_i in range(NQC):
                    push(("k", 2, tc_i), k_chain(2, tc_i))
                bl = attn_unit(1, 1, extra=bl)
                push(("q", 1, 3), q_chain(1, 3))
                push(("q", 2, 0), q_chain(2, 0))
                bl = attn_unit(0, 2, extra=bl)
                push(("q", 2, 1), q_chain(2, 1))
                bl = attn_unit(0, 3, extra=bl)
                for tc_i in range(NQC):
                    push(("k", 3, tc_i), k_chain(3, tc_i))
                bl = attn_unit(1, 2, extra=bl)
                push(("q", 3, 0), q_chain(3, 0))
                push(("q", 3, 1), q_chain(3, 1))
                bl = attn_unit(1, 3, extra=bl)
                push(("q", 2, 2), q_chain(2, 2))
                push(("q", 2, 3), q_chain(2, 3))
                bl = attn_unit(2, 0, extra=bl)
                push(("q", 3, 2), q_chain(3, 2))
                push(("q", 3, 3), q_chain(3, 3))
                bl = attn_unit(2, 1, extra=bl)
                bl = attn_unit(3, 0, extra=bl)
                bl = attn_unit(3, 1, extra=bl)
                # qc0 attn for all pairs completes inside unit (3,1) (it
                # flushes (3,0)'s PV+norm) -> out-proj tc0 can follow
                for ec in range(DC):
                    push(("o", ec, 0), out_chain(ec, 0))
                bl = attn_unit(2, 2, extra=bl, fill=3)
                for ec in range(DC):
                    push(("o", ec, 1), out_chain(ec, 1))
                bl = attn_unit(2, 3, extra=bl, fill=3)
                bl = attn_unit(3, 2, extra=bl, fill=3)
                # fill=0 for the last unit: keep ALL remaining tc2 out-chain
                # steps to bridge the final normalization's latency window
                # (and the HAM clock gate) after the last exp
                bl = attn_unit(3, 3, extra=bl, fill=0)
                for ec in range(DC):
                    push(("o", ec, 2), out_chain(ec, 2))
                for t in bl:          # last unit's PV pairs back-to-back,
                    t()               # then its normalization immediately
                drain_all()           # tc2 chains bridge the norm latency
                for ec in range(DC):
                    push(("o", ec, 3), out_chain(ec, 3))
                drain_all()

    nc.compile()
    return nc


def _prep_in_maps(x, qkv_w, qkv_b, out_w, out_b):
    bf = ml_dtypes.bfloat16
    # xT: [1024 d, 2048 t] -> [128 p, 4 tc, 8 dc, 512] so each tc slice is
    # one contiguous-per-partition DMA
    xTs = []
    for b in range(4):
        xt = x[b].T.astype(bf)                       # [1024, 2048]
        xt = xt.reshape(DC, 128, NQC, QC).transpose(1, 2, 0, 3)
        xTs.append(np.ascontiguousarray(xt))
    wqT, wkT, wvT, owT, bq, bk = [], [], [], [], [], []
    for hh in range(2):
        fsl = slice(hh * FL, (hh + 1) * FL)
        # fc-major: [1024 in, 512 out] -> [128 p, 4 fc, 8 dc, 128 f]
        wqt = qkv_w[0:D][fsl].T.astype(bf)           # [1024 in, 512 out]
        wqT.append(np.ascontiguousarray(
            wqt.reshape(DC, 128, FC, 128).transpose(1, 2, 0, 3)))
        wkt = qkv_w[D:2 * D][fsl].T.astype(bf)
        wkT.append(np.ascontiguousarray(
            wkt.reshape(DC, 128, FC, 128).transpose(1, 2, 0, 3)))
        wvt = qkv_w[2 * D:3 * D][fsl].T.astype(bf)   # [1024 in, 512 out]
        wvT.append(np.ascontiguousarray(
            wvt.reshape(DC, 128, FL).transpose(1, 0, 2)))
        ow = out_w.T[fsl].astype(bf)                 # [512 f, 1024 e]
        owT.append(np.ascontiguousarray(
            ow.reshape(FC, 128, D).transpose(1, 0, 2)))
        bq.append(np.ascontiguousarray(
            qkv_b[0:D][fsl].reshape(FC, 128).T).astype(np.float32))
        bk.append(np.ascontiguousarray(
            qkv_b[D:2 * D][fsl].reshape(FC, 128).T).astype(np.float32))

    in_maps = []
    for i in range(N_CORES):
        b, hh = i // 2, i % 2
        in_maps.append(dict(xT=xTs[b], wqT=wqT[hh], wkT=wkT[hh],
                            wvT=wvT[hh], owT=owT[hh], bq=bq[hh], bk=bk[hh]))
    return in_maps


def run(x, qkv_w, qkv_b, out_w, out_b, trace=False):
    if trace:
        _install_ntff_shim()
    if "nc" not in _CACHE:
        _CACHE["nc"] = build()
    nc = _CACHE["nc"]
    x = np.asarray(x, np.float32)
    qkv_w = np.asarray(qkv_w, np.float32)
    qkv_b = np.asarray(qkv_b, np.float32)
    out_w = np.asarray(out_w, np.float32)
    out_b = np.asarray(out_b, np.float32)
    in_maps = _prep_in_maps(x, qkv_w, qkv_b, out_w, out_b)
    res = run_bass_kernel_spmd(nc, in_maps, core_ids=list(range(N_CORES)),
                               trace=trace)
    # host: sum the two head-half partials per batch, add bv-folded bias
    ob_eff = (out_b + out_w @ qkv_b[2 * D:3 * D]).astype(np.float32)
    out = np.empty((4, NT, D), np.float32)
    for b in range(4):
        acc = (res.results[2 * b]["outT"].astype(np.float32)
               + res.results[2 * b + 1]["outT"].astype(np.float32))
        out[b] = acc.T + ob_eff
    return out, res


def kernel(**inputs):
    out, _ = run(**inputs)
    return out


# revision 17
# speedup vs baseline: 1.0443x; 1.0319x over previous
"""Multi-head attention (b=4, n=2048, dim=1024, heads=16, hd=64) on 8 TRN2
NeuronCores.

Sharding: core i = (batch b = i//2, head-half hh = i%2). Each core computes
Q/K/V projections for its 8 heads only (column-split QKV — no duplicated
K/V work), full 2048x2048 attention for those heads, and a row-split
out-projection partial; the host sums the two partials per batch and adds
the (bv-folded) output bias.

Device layouts (feature-major, partition dim first):
  xT   [128, 4 tc, 8 dc, 512]  x^T, d-chunked
  wq/wk [128, 4 fc, 8 dc, 128] fc-major so the critical fc0 slice is one DMA
  qT   [128, 4 fc, 2048 t]   Q^T local features (head pair p = chunk p)
  kT   [128, 4 fc, 2048 t]   K^T
  v    [128, 16 tt, 8 h, 64] V token-major per head
  S^T  psum [128 k, 2 h, 512 q] per k-tile: even head rows 0:64, odd 64:128
       of the PE array (tile_position row groups -> concurrent matmuls)
  P~   exp(S^T/8) bf16
  PV   po[128, 512] psum: even head d rows 0:64 (col groups 0-1), odd head
       rows 64:128 (col groups 2-3) -> the two M=64 matmuls run CONCURRENT
       in the PE's column tiles, halving PV cost vs the M=65 serial pair
  U    [128, 2, 512] bf16: running elementwise sum of P~ over k-tiles (DVE);
       softmax denominators = ones.T @ U via two tiny M=1 matmuls per unit
  attn [128, 4 fc, 2048 t]   normalized, head-concat feature-major
  outT [1024 e, 2048 t] bf16 partial (host sums core pairs in f32, adds bias)

Schedule: a stream of 16 (pair, q-chunk) units x 16 k-tile slots. Each slot
emits the two row-tiled score matmuls + exp + previous slot's PV, plus
"filler" projection matmuls popped from a deadline-ordered queue so the PE
never idles long enough for the HAM clock gate to re-throttle.

Normalization reshapes the 16 softmax-sum rows into [128, 8] (DRAM bounce)
before the reciprocal: DVE reciprocal is ~8 cyc/element *per partition lane*,
so a [2, 512] layout costs 4096 cycles while [128, 8] costs 64.
"""
import sys

sys.path.insert(0, "/opt/trn_rl_repo")

from collections import deque

import numpy as np
import ml_dtypes

import concourse.bass as bass
import concourse.tile as tile
from concourse import bacc, mybir
from concourse.bass_utils import run_bass_kernel_spmd

BF16 = mybir.dt.bfloat16
F32 = mybir.dt.float32
EXP = mybir.ActivationFunctionType.Exp
MULT = mybir.AluOpType.mult
ADD = mybir.AluOpType.add

D = 1024          # model dim
DC = 8            # d chunks of 128
NT = 2048         # tokens per core (q and k)
FL = 512          # local features (8 heads)
FC = 4            # local feature chunks of 128
NH = 8            # local heads
NP = 4            # local head pairs
HD = 64           # head dim
QC = 512          # q chunk (psum free)
NQC = 4           # q chunks
NKT = 16          # k tiles of 128
SB = 2            # heads per score psum tile (even/odd)
N_CORES = 8

_CACHE = {}


def _install_ntff_shim():
    """The agent image's ``antenv`` lacks ``axon_hooks``, so concourse's
    trace=True path can't find the NTFF profile hook even though
    ``libaxon_pjrt.so`` supports it. Recreate the glue (same contract as
    trn_boot's ``_ntff_profile_via_ctypes``)."""
    import types
    import ctypes
    import contextlib

    if "antenv.axon_hooks" in sys.modules:
        return
    so_path = "/opt/axon/libaxon_pjrt.so"
    try:
        lib = ctypes.CDLL(so_path)
        if not hasattr(lib, "axon_start_nrt_profile"):
            return
    except OSError:
        return
    lib.axon_start_nrt_profile.argtypes = [ctypes.POINTER(ctypes.c_int64),
                                           ctypes.c_size_t]
    lib.axon_start_nrt_profile.restype = ctypes.c_int64
    lib.axon_stop_nrt_profile.argtypes = [ctypes.c_char_p]
    lib.axon_stop_nrt_profile.restype = ctypes.c_int64

    @contextlib.contextmanager
    def _hook(output_dir, device_ids):
        import jax
        jax.devices()
        if device_ids:
            ids = (ctypes.c_int64 * len(device_ids))(*device_ids)
            rc = lib.axon_start_nrt_profile(ids, len(device_ids))
        else:
            rc = lib.axon_start_nrt_profile(None, 0)
        if rc != 0:
            raise RuntimeError(f"axon_start_nrt_profile rc={rc}")
        try:
            yield
        finally:
            n = lib.axon_stop_nrt_profile(str(output_dir).encode())
            print(f"ntff profile: {n} file(s) written to {output_dir}",
                  file=sys.stderr)

    mod = types.ModuleType("antenv.axon_hooks")
    _h = [_hook]
    mod.set_axon_ntff_profile_hook = lambda h: _h.__setitem__(0, h)
    mod.get_axon_ntff_profile_hook = lambda: _h[0]
    sys.modules["antenv.axon_hooks"] = mod
    import antenv
    antenv.axon_hooks = mod


def build():
    nc = bacc.Bacc("TRN2", target_bir_lowering=False, debug=False,
                   num_devices=N_CORES)

    xT_d = nc.dram_tensor("xT", [128, NQC, DC, QC], BF16,
                          kind="ExternalInput")
    wq_d = nc.dram_tensor("wqT", [128, FC, DC, 128], BF16,
                          kind="ExternalInput")
    wk_d = nc.dram_tensor("wkT", [128, FC, DC, 128], BF16,
                          kind="ExternalInput")
    wv_d = nc.dram_tensor("wvT", [128, DC, FL], BF16, kind="ExternalInput")
    ow_d = nc.dram_tensor("owT", [128, FC, D], BF16, kind="ExternalInput")
    bq_d = nc.dram_tensor("bq", [128, FC], F32, kind="ExternalInput")
    bk_d = nc.dram_tensor("bk", [128, FC], F32, kind="ExternalInput")
    out_d = nc.dram_tensor("outT", [D, NT], BF16, kind="ExternalOutput")

    with tile.TileContext(nc) as tc:
        with tc.tile_pool(name="persist", bufs=1) as persist:
            kT = persist.tile([128, FC, NT], BF16)
            qT = persist.tile([128, FC, NT], BF16)
            v = persist.tile([128, NKT, NH, HD], BF16)
            attn = persist.tile([128, FC, NT], BF16)
            bq_sb = persist.tile([128, FC], F32)
            bk_sb = persist.tile([128, FC], F32)
            one1 = persist.tile([128, 1], BF16)
            nc.vector.memset(one1, 1.0)
            warm = persist.tile([128, 1], F32)
            nc.vector.memset(warm, 0.0)

            # PSUM budget (8 banks): ps_acc 2x[128,512] proj/out/sums
            # accumulators, ps_s 2x[128,2,512] scores, ps_o 2x[128,512] PV.
            with tc.tile_pool(name="w1", bufs=1) as w1, \
                 tc.tile_pool(name="xpool", bufs=1) as xpool, \
                 tc.tile_pool(name="ppool", bufs=17) as ppool, \
                 tc.tile_pool(name="usum", bufs=2) as usum, \
                 tc.tile_pool(name="nrm", bufs=2) as nrm, \
                 tc.tile_pool(name="fout", bufs=3) as fout, \
                 tc.tile_pool(name="drpool", bufs=4, space="DRAM") as drpool, \
                 tc.tile_pool(name="ps_acc", bufs=2, space="PSUM") as ps_acc, \
                 tc.tile_pool(name="ps_s", bufs=2, space="PSUM") as ps_s, \
                 tc.tile_pool(name="ps_o", bufs=2, space="PSUM") as ps_o:
                xT = xpool.tile([128, NQC, DC, QC], BF16)
                wq = w1.tile([128, FC, DC, 128], BF16, tag="wq")
                wk = w1.tile([128, FC, DC, 128], BF16, tag="wk")
                wv = w1.tile([128, DC, FL], BF16, tag="wv")
                ow = w1.tile([128, FC, D], BF16, tag="ow")

                # Three queues in parallel, critical-first: the first K chain
                # needs only wk fc0 (one 256KB transfer) + xT tc0 (d-chunked,
                # progressive), then Q(0,0) needs wq fc0.
                nc.scalar.dma_start(out=bq_sb, in_=bq_d.ap())
                nc.scalar.dma_start(out=bk_sb, in_=bk_d.ap())
                nc.scalar.dma_start(out=wk[:, 0], in_=wk_d.ap()[:, 0])
                nc.scalar.dma_start(out=wq[:, 0], in_=wq_d.ap()[:, 0])
                # dummy exp pulls the ACT_TABLE_LOAD (~2.7us, blocks the
                # scalar sequencer) off the critical path — issued after the
                # critical weight DMAs so the table load overlaps transfers
                nc.scalar.activation(warm, warm, EXP)
                nc.sync.dma_start(out=xT[:, 0, 0:1, :],
                                  in_=xT_d.ap()[:, 0, 0:1, :])
                nc.sync.dma_start(out=xT[:, 0, 1:2, :],
                                  in_=xT_d.ap()[:, 0, 1:2, :])
                nc.sync.dma_start(out=xT[:, 0, 2:4, :],
                                  in_=xT_d.ap()[:, 0, 2:4, :])
                nc.sync.dma_start(out=xT[:, 0, 4:6, :],
                                  in_=xT_d.ap()[:, 0, 4:6, :])
                nc.sync.dma_start(out=xT[:, 0, 6:8, :],
                                  in_=xT_d.ap()[:, 0, 6:8, :])
                nc.scalar.dma_start(out=wk[:, 1], in_=wk_d.ap()[:, 1])
                nc.scalar.dma_start(out=wq[:, 1], in_=wq_d.ap()[:, 1])
                nc.gpsimd.dma_start(out=wv[:, 0:4, :], in_=wv_d.ap()[:, 0:4, :])
                nc.gpsimd.dma_start(out=wv[:, 4:8, :], in_=wv_d.ap()[:, 4:8, :])
                for h in (slice(0, 4), slice(4, 8)):
                    nc.sync.dma_start(out=xT[:, 1, h, :],
                                      in_=xT_d.ap()[:, 1, h, :])
                for fcw in (2, 3):
                    nc.gpsimd.dma_start(out=wk[:, fcw], in_=wk_d.ap()[:, fcw])
                    nc.gpsimd.dma_start(out=wq[:, fcw], in_=wq_d.ap()[:, fcw])
                nc.sync.dma_start(out=xT[:, 2, :, :], in_=xT_d.ap()[:, 2, :, :])
                nc.sync.dma_start(out=xT[:, 3, :, :], in_=xT_d.ap()[:, 3, :, :])
                nc.gpsimd.dma_start(out=ow, in_=ow_d.ap())

                # ---- projection chains (8 matmuls + epilogue each) ----
                def k_chain(fc, tc_i):
                    tsl = slice(tc_i * QC, (tc_i + 1) * QC)
                    ps = ps_acc.tile([128, QC], F32, tag="ps")
                    for dc in range(DC):
                        yield nc.tensor.matmul(
                            ps, lhsT=wk[:, fc, dc, :],
                            rhs=xT[:, tc_i, dc, :],
                            start=(dc == 0), stop=(dc == DC - 1))
                    yield nc.vector.tensor_scalar_add(
                        kT[:, fc, tsl], ps, bk_sb[:, fc:fc + 1])

                def q_chain(fc, tc_i):
                    tsl = slice(tc_i * QC, (tc_i + 1) * QC)
                    ps = ps_acc.tile([128, QC], F32, tag="ps")
                    for dc in range(DC):
                        yield nc.tensor.matmul(
                            ps, lhsT=wq[:, fc, dc, :],
                            rhs=xT[:, tc_i, dc, :],
                            start=(dc == 0), stop=(dc == DC - 1))
                    yield nc.vector.tensor_scalar_add(
                        qT[:, fc, tsl], ps, bq_sb[:, fc:fc + 1])

                def v_chain(tt):
                    ps = ps_acc.tile([128, QC], F32, tag="ps")
                    for dc in range(DC):
                        yield nc.tensor.matmul(
                            ps,
                            lhsT=xT[:, tt // 4, dc,
                                    (tt % 4) * 128:(tt % 4) * 128 + 128],
                            rhs=wv[:, dc, :],
                            start=(dc == 0), stop=(dc == DC - 1))
                    yield nc.vector.tensor_copy(
                        out=v[:, tt, :, :],
                        in_=ps.rearrange("p (h d) -> p h d", d=HD))

                def out_chain(ec, tc_i):
                    tsl = slice(tc_i * QC, (tc_i + 1) * QC)
                    ps = ps_acc.tile([128, QC], F32, tag="ps")
                    for fc in range(FC):
                        yield nc.tensor.matmul(
                            ps, lhsT=ow[:, fc, ec * 128:(ec + 1) * 128],
                            rhs=attn[:, fc, tsl],
                            start=(fc == 0), stop=(fc == FC - 1))
                    fo = fout.tile([128, QC], BF16, tag="fo")
                    # tc2/tc3 chains run in the tail where ACT is idle and
                    # DVE is busy with the final normalizations
                    if tc_i >= 2:
                        yield nc.scalar.activation(
                            fo, ps, mybir.ActivationFunctionType.Copy)
                    else:
                        yield nc.vector.tensor_copy(out=fo, in_=ps)
                    if tc_i == 3:
                        eng = (nc.gpsimd, nc.sync, nc.scalar)[ec % 3]
                    else:
                        eng = nc.gpsimd
                    yield eng.dma_start(
                        out=out_d.ap()[ec * 128:(ec + 1) * 128, tsl], in_=fo)

                # Deadline-ordered filler queue of (key, generator); attn
                # units pop a couple of steps per k-tile slot to keep the PE
                # dense while ACT owns the critical path.  Correctness rule:
                # everything a unit's own matmuls READ must be fully emitted
                # before the unit emits them — require() force-drains those.
                filler = deque()
                done_keys = set()

                def push(key, gen):
                    filler.append((key, gen))

                def drain(n):
                    for _ in range(n):
                        if not filler:
                            return
                        key, gen = filler[0]
                        try:
                            next(gen)
                        except StopIteration:
                            done_keys.add(key)
                            filler.popleft()

                def drain_all():
                    while filler:
                        drain(1)

                def require(*keys):
                    while any(k not in done_keys for k in keys):
                        assert filler, f"missing filler chains: {keys}"
                        drain(1)

                def attn_unit(p, qc, first=False, fill=2, extra=()):
                    # Cascaded schedule: EVERY unit defers its 16 PV matmul
                    # pairs + normalization into the NEXT unit's slots (the
                    # `extra` thunks, flushed three per slot so the previous
                    # unit's normalization fires ~10 slots before the unit
                    # boundary — out-proj chains gate on its attn write).
                    require(("k", p, 0), ("q", p, qc))
                    if not first:
                        require(*[("v", tt) for tt in range(NKT)])
                    he, ho = 2 * p, 2 * p + 1
                    qsl = slice(qc * QC, (qc + 1) * QC)
                    # one [128, 512] PSUM tile: even head d on partitions
                    # 0:64 (PE col groups 0-1), odd head on 64:128 (groups
                    # 2-3) -> the two PV matmuls execute concurrently
                    po = ps_o.tile([128, QC], F32, tag="po")
                    U = usum.tile([128, SB, QC], BF16, tag="U")

                    def pv(pt, kt):
                        nc.tensor.matmul(
                            po[0:HD, :], lhsT=v[:, kt, he, :], rhs=pt[:, 0, :],
                            start=(kt == 0), stop=(kt == NKT - 1))
                        nc.tensor.matmul(
                            po[HD:128, :], lhsT=v[:, kt, ho, :],
                            rhs=pt[:, 1, :],
                            start=(kt == 0), stop=(kt == NKT - 1))

                    extra = deque(extra)
                    backlog = []
                    for kt in range(NKT):
                        if kt % 4 == 0 and kt > 0:
                            require(("k", p, kt // 4))
                        ss = ps_s.tile([128, SB, QC], F32, tag="ss")
                        for j in range(SB):
                            hi = j * 64
                            nc.tensor.matmul(
                                ss[:, j, :],
                                lhsT=kT[hi:hi + HD, p,
                                        kt * 128:(kt + 1) * 128],
                                rhs=qT[hi:hi + HD, p, qsl],
                                start=True, stop=True)
                        pt = ppool.tile([128, SB, QC], BF16, tag="pt",
                                        bufs=19)
                        nc.scalar.activation(pt, ss, EXP, scale=0.125)
                        # running softmax-denominator accumulation (bf16,
                        # 2x DVE mode) — replaces the ones-column in V so
                        # the PV pair can column-tile at M=64
                        if kt == 0:
                            nc.vector.tensor_copy(out=U, in_=pt)
                        else:
                            nc.vector.tensor_tensor(out=U, in0=U, in1=pt,
                                                    op=ADD)
                        backlog.append((pt, kt))
                        for _ in range(3):
                            if extra:
                                extra.popleft()()
                        drain(fill)
                    while extra:
                        extra.popleft()()

                    def norm():
                        _norm(p, qc, po, U)

                    return ([lambda a=a, b=b: pv(a, b)
                             for a, b in backlog] + [norm])

                def _norm(p, qc, po, U):
                    # softmax denominators: ones.T @ U via two M=1 matmuls
                    # (psum rows 0 and 32 -> different PE column groups),
                    # then reshape to [128, 8] (DRAM bounce), reciprocal,
                    # DRAM-bounce the partition broadcast, one full-width
                    # bf16 multiply writes both heads' attn directly.
                    qsl = slice(qc * QC, (qc + 1) * QC)
                    su = ps_acc.tile([128, QC], F32, tag="ps")
                    nc.tensor.matmul(su[0:1, :], lhsT=one1, rhs=U[:, 0, :],
                                     start=True, stop=True)
                    nc.tensor.matmul(su[32:33, :], lhsT=one1, rhs=U[:, 1, :],
                                     start=True, stop=True)
                    sums = nrm.tile([33, QC], BF16, tag="sums")
                    nc.vector.tensor_copy(out=sums, in_=su[0:33, :])
                    pv_sb = nrm.tile([128, QC], BF16, tag="pv_sb", bufs=3)
                    nc.vector.tensor_copy(out=pv_sb, in_=po)
                    ds = drpool.tile([2, QC], BF16, tag="ds")
                    nc.sync.dma_start(out=ds[0:1, :], in_=sums[0:1, :])
                    nc.sync.dma_start(out=ds[1:2, :], in_=sums[32:33, :])
                    sr = nrm.tile([128, 8], BF16, tag="sr")
                    nc.sync.dma_start(
                        out=sr,
                        in_=bass.AP(tensor=ds.tensor, offset=ds.offset,
                                    ap=[[8, 128], [1, 8]]))
                    rc = nrm.tile([128, 8], BF16, tag="rc")
                    with nc.allow_low_precision(
                            reason="bf16 softmax denominators; rel-err "
                                   "budget 2e-2 >> bf16 rounding"):
                        nc.vector.reciprocal(rc, sr)
                    dr = drpool.tile([2, QC], BF16, tag="dr")
                    nc.sync.dma_start(
                        out=bass.AP(tensor=dr.tensor, offset=dr.offset,
                                    ap=[[8, 128], [1, 8]]),
                        in_=rc)
                    bc = nrm.tile([128, QC], BF16, tag="bc")
                    nc.sync.dma_start(
                        out=bc[0:HD, :],
                        in_=bass.AP(tensor=dr.tensor, offset=dr.offset,
                                    ap=[[0, HD], dr.ap[-1]]))
                    nc.sync.dma_start(
                        out=bc[HD:128, :],
                        in_=bass.AP(tensor=dr.tensor,
                                    offset=dr.offset + QC,
                                    ap=[[0, HD], dr.ap[-1]]))
                    nc.vector.tensor_tensor(
                        out=attn[:, p, qsl], in0=pv_sb, in1=bc, op=MULT)

                # ---- emission ----
                # preamble: only K(0, tc0) + Q(0, qc0) gate the first scores
                push(("k", 0, 0), k_chain(0, 0))
                push(("q", 0, 0), q_chain(0, 0))
                require(("k", 0, 0), ("q", 0, 0))

                # unit 1's early fillers prefer tc0-data chains (xT tc1-3
                # are still in flight); V chains follow once wv lands
                push(("k", 1, 0), k_chain(1, 0))
                push(("q", 1, 0), q_chain(1, 0))
                for tc_i in range(1, NQC):
                    push(("k", 0, tc_i), k_chain(0, tc_i))
                for tt in range(NKT):
                    push(("v", tt), v_chain(tt))
                push(("q", 0, 1), q_chain(0, 1))
                for tc_i in range(1, NQC):
                    push(("k", 1, tc_i), k_chain(1, tc_i))
                bl = attn_unit(0, 0, first=True, fill=10)

                push(("q", 1, 1), q_chain(1, 1))
                push(("q", 0, 2), q_chain(0, 2))
                bl = attn_unit(0, 1, extra=bl, fill=3)
                push(("q", 0, 3), q_chain(0, 3))
                push(("q", 1, 2), q_chain(1, 2))
                bl = attn_unit(1, 0, extra=bl)
                for tc_i in range(NQC):
                    push(("k", 2, tc_i), k_chain(2, tc_i))
                bl = attn_unit(1, 1, extra=bl)
                push(("q", 1, 3), q_chain(1, 3))
                push(("q", 2, 0), q_chain(2, 0))
                bl = attn_unit(0, 2, extra=bl)
                push(("q", 2, 1), q_chain(2, 1))
                bl = attn_unit(0, 3, extra=bl)
                for tc_i in range(NQC):
                    push(("k", 3, tc_i), k_chain(3, tc_i))
                bl = attn_unit(1, 2, extra=bl)
                push(("q", 3, 0), q_chain(3, 0))
                push(("q", 3, 1), q_chain(3, 1))
                bl = attn_unit(1, 3, extra=bl)
                push(("q", 2, 2), q_chain(2, 2))
                push(("q", 2, 3), q_chain(2, 3))
                bl = attn_unit(2, 0, extra=bl)
                push(("q", 3, 2), q_chain(3, 2))
                push(("q", 3, 3), q_chain(3, 3))
                bl = attn_unit(2, 1, extra=bl)
                bl = attn_unit(3, 0, extra=bl)
                bl = attn_unit(3, 1, extra=bl)
                # qc0 attn for all pairs completes inside unit (3,1) (it
                # flushes (3,0)'s PV+norm) -> out-proj tc0 can follow
                for ec in range(DC):
                    push(("o", ec, 0), out_chain(ec, 0))
                bl = attn_unit(2, 2, extra=bl, fill=3)
                for ec in range(DC):
                    push(("o", ec, 1), out_chain(ec, 1))
                bl = attn_unit(2, 3, extra=bl, fill=3)
                bl = attn_unit(3, 2, extra=bl, fill=3)
                # fill=0 for the last unit: keep ALL remaining tc2 out-chain
                # steps to bridge the final normalization's latency window
                # (and the HAM clock gate) after the last exp
                bl = attn_unit(3, 3, extra=bl, fill=0)
                for ec in range(DC):
                    push(("o", ec, 2), out_chain(ec, 2))
                for t in bl:          # last unit's PV pairs back-to-back,
                    t()               # then its normalization immediately
                drain_all()           # tc2 chains bridge the norm latency
                for ec in range(DC):
                    push(("o", ec, 3), out_chain(ec, 3))
                drain_all()

    nc.compile()
    return nc


def _prep_in_maps(x, qkv_w, qkv_b, out_w, out_b):
    bf = ml_dtypes.bfloat16
    # xT: [1024 d, 2048 t] -> [128 p, 4 tc, 8 dc, 512] so each tc slice is
    # one contiguous-per-partition DMA
    xTs = []
    for b in range(4):
        xt = x[b].T.astype(bf)                       # [1024, 2048]
        xt = xt.reshape(DC, 128, NQC, QC).transpose(1, 2, 0, 3)
        xTs.append(np.ascontiguousarray(xt))
    wqT, wkT, wvT, owT, bq, bk = [], [], [], [], [], []
    for hh in range(2):
        fsl = slice(hh * FL, (hh + 1) * FL)
        # fc-major: [1024 in, 512 out] -> [128 p, 4 fc, 8 dc, 128 f]
        wqt = qkv_w[0:D][fsl].T.astype(bf)           # [1024 in, 512 out]
        wqT.append(np.ascontiguousarray(
            wqt.reshape(DC, 128, FC, 128).transpose(1, 2, 0, 3)))
        wkt = qkv_w[D:2 * D][fsl].T.astype(bf)
        wkT.append(np.ascontiguousarray(
            wkt.reshape(DC, 128, FC, 128).transpose(1, 2, 0, 3)))
        wvt = qkv_w[2 * D:3 * D][fsl].T.astype(bf)   # [1024 in, 512 out]
        wvT.append(np.ascontiguousarray(
            wvt.reshape(DC, 128, FL).transpose(1, 0, 2)))
        ow = out_w.T[fsl].astype(bf)                 # [512 f, 1024 e]
        owT.append(np.ascontiguousarray(
            ow.reshape(FC, 128, D).transpose(1, 0, 2)))
        bq.append(np.ascontiguousarray(
            qkv_b[0:D][fsl].reshape(FC, 128).T).astype(np.float32))
        bk.append(np.ascontiguousarray(
            qkv_b[D:2 * D][fsl].reshape(FC, 128).T).astype(np.float32))

    in_maps = []
    for i in range(N_CORES):
        b, hh = i // 2, i % 2
        in_maps.append(dict(xT=xTs[b], wqT=wqT[hh], wkT=wkT[hh],
                            wvT=wvT[hh], owT=owT[hh], bq=bq[hh], bk=bk[hh]))
    return in_maps


def run(x, qkv_w, qkv_b, out_w, out_b, trace=False):
    if trace:
        _install_ntff_shim()
    if "nc" not in _CACHE:
        _CACHE["nc"] = build()
    nc = _CACHE["nc"]
    x = np.asarray(x, np.float32)
    qkv_w = np.asarray(qkv_w, np.float32)
    qkv_b = np.asarray(qkv_b, np.float32)
    out_w = np.asarray(out_w, np.float32)
    out_b = np.asarray(out_b, np.float32)
    in_maps = _prep_in_maps(x, qkv_w, qkv_b, out_w, out_b)
    res = run_bass_kernel_spmd(nc, in_maps, core_ids=list(range(N_CORES)),
                               trace=trace)
    # host: sum the two head-half partials per batch, add bv-folded bias
    ob_eff = (out_b + out_w @ qkv_b[2 * D:3 * D]).astype(np.float32)
    out = np.empty((4, NT, D), np.float32)
    for b in range(4):
        acc = (res.results[2 * b]["outT"].astype(np.float32)
               + res.results[2 * b + 1]["outT"].astype(np.float32))
        out[b] = acc.T + ob_eff
    return out, res


def kernel(**inputs):
    out, _ = run(**inputs)
    return out
